# revision 7
# baseline (speedup 1.0000x reference)
"""DGCNN (dynamic-graph edge conv) Trainium2 Bass kernel.

Strategy (per sample, one NeuronCore each; B=4 samples on cores 0-3):
  - Each edge_conv layer:  key[i,j] = <f_i, f_j> - |f_j|^2/2  (monotone
    transform of -dist^2, so top-k indices match the reference), computed
    by the PE with an extra contraction row carrying -|f_j|^2/2.
  - top-20 per row via DVE max8 / max_index / match_replace (3 rounds).
  - Linearized edge conv:  h_ijk = a_i + b_j  with a = (Wc - Wn) @ f,
    b = Wn @ f (signs of gamma folded into a,b so max over k commutes
    with the GroupNorm affine).
  - Neighbour gather of b columns with GPSIMD ap_gather (indices are
    replicated into each 16-partition group via tiny PE matmuls against
    an identity matrix).
  - GroupNorm stats accumulated with ScalarE accum_out; finalize with a
    per-partition affine + leaky relu.
  - Aggregation conv + global max/mean pool + MLP head on PE/DVE/ACT.
"""

import os
from contextlib import ExitStack

import numpy as np

import concourse.bacc as bacc
import concourse.bass as bass
import concourse.mybir as mybir
import concourse.tile as tile

F32 = mybir.dt.float32
I16 = mybir.dt.int16
U16 = mybir.dt.uint16

P = 128
K = 20
G = 8
EPS = 1e-5
NEG = -3.0e38

# (Cin, Cout, dual) per edge conv layer. dual=True packs two copies of the
# 64 output channels into 128 partitions so one ap_gather covers all 20
# neighbours (top half k0-15, bottom half k16-19).
LAYERS = [(8, 64, True), (64, 64, True), (64, 128, False)]


def _sf(x):
    return np.ascontiguousarray(x, dtype=np.float32)


def host_prep(inputs, N):
    """Derive all constant tensors shipped to every core."""
    c = {}
    c["eye"] = _sf(np.eye(P))

    Ws = [(inputs["W1"], inputs["g1"], inputs["b1"]),
          (inputs["W2"], inputs["g2"], inputs["b2"]),
          (inputs["W3"], inputs["g3"], inputs["b3"])]
    for l, ((W, g, b), (Cin, Cout, dual)) in enumerate(zip(Ws, LAYERS)):
        Wc, Wn = W[:, :Cin], W[:, Cin:]
        sign = np.where(g >= 0, 1.0, -1.0).astype(np.float32)
        A = ((Wc - Wn) * sign[:, None]).T  # (Cin, Cout), sign folded
        Bm = (Wn * sign[:, None]).T
        ncols = 128
        Kr = 32 if l == 0 else Cin  # ones/halfsq row (32-aligned for DVE/ACT)
        Aw = np.zeros((Kr + 1, ncols), np.float32)
        Bw = np.zeros((Kr + 1, ncols), np.float32)
        if dual:
            Aw[:Cin, :Cout] = A
            Aw[:Cin, Cout:2 * Cout] = A
            Bw[:Cin, :Cout] = Bm
            Bw[:Cin, Cout:2 * Cout] = Bm
        else:
            Aw[:Cin, :Cout] = A
            Bw[:Cin, :Cout] = Bm
        c[f"Aw{l}"] = _sf(Aw)
        c[f"Bw{l}"] = _sf(Bw)
        # group indicator matrices
        cpg = Cout // G
        gh = np.zeros((Cout, G), np.float32)
        for gg in range(G):
            gh[gg * cpg:(gg + 1) * cpg, gg] = 1.0
        c[f"GHs{l}"] = _sf(gh * sign[:, None])
        c[f"GHu{l}"] = _sf(gh)
        c[f"GT{l}"] = _sf(gh.T)
        c[f"absg{l}"] = _sf(np.abs(g)[:, None])
        c[f"g{l}"] = _sf(g[:, None])
        c[f"beta{l}"] = _sf(b[:, None])

    # aggregation conv Wa (256, 256) split into 2 output halves x 3 in chunks
    Wa = inputs["Wa"]
    chunks = [(0, 64), (64, 128), (128, 256)]
    for h in range(2):
        for ci, (s, e) in enumerate(chunks):
            c[f"WaT{h}{ci}"] = _sf(Wa[h * 128:(h + 1) * 128, s:e].T)
    ga, ba = inputs["ga"], inputs["ba"]
    for h in range(2):
        gh = np.zeros((128, G), np.float32)
        for gg in range(4):
            gh[gg * 32:(gg + 1) * 32, 4 * h + gg] = 1.0
        # only 4 groups per half are referenced; build GT mapping back
        c[f"GHa{h}"] = _sf(gh)
        gt = np.zeros((G, 128), np.float32)
        for gg in range(4):
            gt[4 * h + gg, gg * 32:(gg + 1) * 32] = 1.0
        c[f"GTa{h}"] = _sf(gt)
        c[f"ga{h}"] = _sf(ga[h * 128:(h + 1) * 128][:, None])
        c[f"betaa{h}"] = _sf(ba[h * 128:(h + 1) * 128][:, None])

    def gn_consts(prefix, C, gamma, beta):
        cpg = C // G
        gh = np.zeros((C, G), np.float32)
        for gg in range(G):
            gh[gg * cpg:(gg + 1) * cpg, gg] = 1.0
        c[f"GH{prefix}"] = _sf(gh)
        c[f"GT{prefix}"] = _sf(gh.T)
        c[f"g{prefix}"] = _sf(gamma[:, None])
        c[f"beta{prefix}"] = _sf(beta[:, None])

    gn_consts("c1", 64, inputs["gc1"], inputs["bc1"])
    gn_consts("c2", 64, inputs["gc2"], inputs["bc2"])
    gn_consts("z2", 128, inputs["gs2"], inputs["bs2"])
    gn_consts("z3", 64, inputs["gs3"], inputs["bs3"])
    # z1 (256 ch) as two halves of 128, groups of 32
    gs1, bs1 = inputs["gs1"], inputs["bs1"]
    for h in range(2):
        gh = np.zeros((128, G), np.float32)
        gt = np.zeros((G, 128), np.float32)
        for gg in range(4):
            gh[gg * 32:(gg + 1) * 32, 4 * h + gg] = 1.0
            gt[4 * h + gg, gg * 32:(gg + 1) * 32] = 1.0
        c[f"GHz1{h}"] = _sf(gh)
        c[f"GTz1{h}"] = _sf(gt)
        c[f"gz1{h}"] = _sf(gs1[h * 128:(h + 1) * 128][:, None])
        c[f"betaz1{h}"] = _sf(bs1[h * 128:(h + 1) * 128][:, None])

    c["Wc1T"] = _sf(inputs["Wc1"].T)   # (2, 64)
    c["Wc2T"] = _sf(inputs["Wc2"].T)   # (64, 64)
    # Ws1 (256, 576): z order [max0(128) max1(128) mean0(128) mean1(128) c2(64)]
    Ws1 = inputs["Ws1"]
    zchunks = [(0, 128), (128, 256), (256, 384), (384, 512), (512, 576)]
    for h in range(2):
        for ci, (s, e) in enumerate(zchunks):
            c[f"Ws1T{h}{ci}"] = _sf(Ws1[h * 128:(h + 1) * 128, s:e].T)
    Ws2 = inputs["Ws2"]
    for h in range(2):
        c[f"Ws2T{h}"] = _sf(Ws2[:, h * 128:(h + 1) * 128].T)
    c["Ws3T"] = _sf(inputs["Ws3"].T)   # (128, 64)
    c["Ws4T"] = _sf(inputs["Ws4"].T)   # (64, 2)
    c["bs4"] = _sf(inputs["bs4"][:, None])
    c["neghalf"] = _sf(np.full((64, 1), -0.5))
    return c


def const_shapes(N):
    """Shapes of every constant input (dict name -> shape)."""
    fake = {
        "W1": np.zeros((64, 16)), "g1": np.zeros(64), "b1": np.zeros(64),
        "W2": np.zeros((64, 128)), "g2": np.zeros(64), "b2": np.zeros(64),
        "W3": np.zeros((128, 128)), "g3": np.zeros(128), "b3": np.zeros(128),
        "Wa": np.zeros((256, 256)), "ga": np.zeros(256), "ba": np.zeros(256),
        "Wc1": np.zeros((64, 2)), "gc1": np.zeros(64), "bc1": np.zeros(64),
        "Wc2": np.zeros((64, 64)), "gc2": np.zeros(64), "bc2": np.zeros(64),
        "Ws1": np.zeros((256, 576)), "gs1": np.zeros(256), "bs1": np.zeros(256),
        "Ws2": np.zeros((128, 256)), "gs2": np.zeros(128), "bs2": np.zeros(128),
        "Ws3": np.zeros((64, 128)), "gs3": np.zeros(64), "bs3": np.zeros(64),
        "Ws4": np.zeros((2, 64)), "bs4": np.zeros(2),
    }
    return {k: v.shape for k, v in host_prep(fake, N).items()}


def build_nc(N, num_devices=1):
    """Build the single-core SPMD program (one full sample per core)."""
    nc = bacc.Bacc("TRN2", target_bir_lowering=False, debug=False,
                   num_devices=num_devices)
    nblk = N // P

    dram = {}
    dram["x"] = nc.dram_tensor("x", [8, N], F32, kind="ExternalInput")
    dram["cond"] = nc.dram_tensor("cond", [2, 1], F32, kind="ExternalInput")
    for name, shape in const_shapes(N).items():
        dram[name] = nc.dram_tensor(name, list(shape), F32, kind="ExternalInput")
    out_d = nc.dram_tensor("out", [2, 1], F32, kind="ExternalOutput")

    with tile.TileContext(nc) as tc:
        with ExitStack() as ctx:
            emit(ctx, tc, nc, dram, out_d, N, nblk)
    nc.compile()
    return nc


def emit(ctx, tc, nc, dram, out_d, N, nblk):
    cpool = ctx.enter_context(tc.tile_pool(name="consts", bufs=1))
    fpool = ctx.enter_context(tc.tile_pool(name="feat", bufs=1))
    wpool = ctx.enter_context(tc.tile_pool(name="work", bufs=2))
    spool = ctx.enter_context(tc.tile_pool(name="small", bufs=2))
    pk = ctx.enter_context(tc.tile_pool(name="pkt", bufs=1, space="PSUM"))
    pidxp = ctx.enter_context(tc.tile_pool(name="pidx", bufs=2, space="PSUM"))
    psp = ctx.enter_context(tc.tile_pool(name="psmall", bufs=2, space="PSUM"))

    V = nc.vector
    S = nc.scalar
    T = nc.tensor
    Y = nc.sync

    # ---- load constants ----
    cs = {}
    for name, d in dram.items():
        if name in ("x", "cond"):
            continue
        t = cpool.tile(list(d.shape), F32, tag=f"c_{name}")
        Y.dma_start(t[:, :], d[:, :])
        cs[name] = t
    cond_sb = cpool.tile([2, 1], F32, tag="c_cond")
    Y.dma_start(cond_sb[:, :], dram["cond"][:, :])

    # ---- persistent feature tiles ----
    f0 = fpool.tile([33, N], F32, tag="f0")
    f1 = fpool.tile([65, N], F32, tag="f1")
    f2 = fpool.tile([65, N], F32, tag="f2")
    f3 = fpool.tile([128, N], F32, tag="f3")
    V.memset(f0[0:33, :], 0.0)
    Y.dma_start(f0[0:8, :], dram["x"][:, :])
    V.memset(f0[32:33, :], 1.0)
    V.memset(f1[64:65, :], 1.0)
    V.memset(f2[64:65, :], 1.0)
    ftiles = [f0, f1, f2, f3]

    Hf = N // 2  # free-dim half for psum rounds

    def psum_to_sbuf(dst_ap, src_psum, accum=None):
        S.activation(dst_ap, src_psum, mybir.ActivationFunctionType.Identity,
                     accum_out=accum)

    # ================= edge conv layers =================
    for l, (Cin, Cout, dual) in enumerate(LAYERS):
        fl = ftiles[l]
        fn = ftiles[l + 1]
        Kr = 32 if l == 0 else Cin
        Kc = Kr + 1

        frhs = fpool.tile([66, N], F32, tag="frhs")
        if l == 0:
            V.memset(frhs[0:33, :], 0.0)
            Y.dma_start(frhs[0:8, :], dram["x"][:, :])
        else:
            V.tensor_copy(frhs[0:Cin, :], fl[0:Cin, :])

        # -0.5*|f_j|^2 row
        fsq = wpool.tile([Cin, N], F32, tag="keysb")
        S.activation(fsq[:, :], fl[0:Cin, :], mybir.ActivationFunctionType.Square)
        for half in range(2):
            kt = pk.tile([P, Hf], F32, tag="kt")
            for j0 in range(0, Hf, 512):
                j1 = min(j0 + 512, Hf)
                T.matmul(kt[0:1, j0:j1],
                         lhsT=cs["neghalf"][0:Cin, :],
                         rhs=fsq[:, half * Hf + j0: half * Hf + j1])
            psum_to_sbuf(frhs[Kr:Kr + 1, half * Hf:(half + 1) * Hf], kt[0:1, :])

        # b table (gather source) and a table
        bb = fpool.tile([P, N + 1], F32, tag="bb")
        a2 = fpool.tile([P, N], F32, tag="a2")
        for half in range(2):
            kt = pk.tile([P, Hf], F32, tag="kt")
            for j0 in range(0, Hf, 512):
                j1 = min(j0 + 512, Hf)
                sl = slice(half * Hf + j0, half * Hf + j1)
                T.matmul(kt[:, j0:j1],
                         lhsT=cs[f"Bw{l}"][:, :], rhs=frhs[0:Kc, sl])
            psum_to_sbuf(bb[:, half * Hf:(half + 1) * Hf], kt[:, :])
        V.memset(bb[:, N:N + 1], 0.0)
        for half in range(2):
            kt = pk.tile([P, Hf], F32, tag="kt")
            for j0 in range(0, Hf, 512):
                j1 = min(j0 + 512, Hf)
                sl = slice(half * Hf + j0, half * Hf + j1)
                T.matmul(kt[:, j0:j1],
                         lhsT=cs[f"Aw{l}"][:, :], rhs=frhs[0:Kc, sl])
            psum_to_sbuf(a2[:, half * Hf:(half + 1) * Hf], kt[:, :])

        # stat tiles (pre-activation maxes land directly in fn)
        ncols = nblk if dual else 2 * nblk
        st_s = fpool.tile([P, ncols], F32, tag="st_s")
        st_q = fpool.tile([P, ncols], F32, tag="st_q")

        # ---- per row-block ----
        for blk in range(nblk):
            cols = slice(blk * P, (blk + 1) * P)
            lhsT = fl[0:Kc, cols]
            keysb = wpool.tile([P, N], F32, tag="keysb")
            for half in range(2):
                kt = pk.tile([P, Hf], F32, tag="kt")
                for j0 in range(0, Hf, 512):
                    j1 = min(j0 + 512, Hf)
                    sl = slice(half * Hf + j0, half * Hf + j1)
                    T.matmul(kt[:, j0:j1], lhsT=lhsT,
                             rhs=frhs[0:Kc, sl])
                psum_to_sbuf(keysb[:, half * Hf:(half + 1) * Hf], kt[:, :])

            # top-24 (indices of top-20 used)
            Tw = wpool.tile([P, 32], U16, tag="Tw")
            mx8 = wpool.tile([P, 8], F32, tag="mx8")
            for r in range(3):
                V.max(out=mx8[:, :], in_=keysb[:, :])
                V.max_index(Tw[:, r * 8:(r + 1) * 8], mx8[:, :], keysb[:, :])
                if r < 2:
                    V.match_replace(out=keysb[:, :], in_to_replace=mx8[:, :],
                                    in_values=keysb[:, :], imm_value=NEG)

            # index tile patterns: cols 0:64 = A (k0-15 x4), 64:128 = B x4
            T32 = wpool.tile([P, 128], F32, tag="T32")
            V.memset(T32[:, 64:128], float(N))
            for q in range(4):
                V.tensor_copy(T32[:, 16 * q:16 * q + 16], Tw[:, 0:16])
                V.tensor_copy(T32[:, 64 + 16 * q:64 + 16 * q + 4], Tw[:, 16:20])

            pidx = pidxp.tile([P, 256], F32, tag="pidx")
            for q in range(2):
                T.matmul(pidx[64 * q:64 * (q + 1), 0:128],
                         lhsT=T32[:, 0:64], rhs=cs["eye"][:, :])
                T.matmul(pidx[64 * q:64 * (q + 1), 128:256],
                         lhsT=T32[:, 64:128], rhs=cs["eye"][:, :])

            if dual:
                iw = wpool.tile([P, 128], I16, tag="iw")
                V.tensor_copy(iw[0:64, :], pidx[0:64, 0:128])
                V.tensor_copy(iw[64:128, :], pidx[64:128, 128:256])
                gb = wpool.tile([P, 2048], F32, tag="gb")
                nc.gpsimd.ap_gather(gb[:, :], bb[:, :], iw[:, :], channels=128,
                                    num_elems=N + 1, d=1, num_idxs=2048)
                gt_ = gb[0:64, :].rearrange("p (n k) -> p n k", k=16)
                gbv = gb[64:128, :].rearrange("p (n k) -> p n k", k=16)[:, :, 0:4]
                at = a2[0:64, cols].unsqueeze(2).to_broadcast([64, P, 16])
                ab = a2[64:128, cols].unsqueeze(2).to_broadcast([64, P, 4])
                V.tensor_add(gt_, gt_, at)
                V.tensor_add(gbv, gbv, ab)
                mcomb = wpool.tile([P, P], F32, tag="mcomb")
                V.reduce_max(mcomb[0:64, :], gt_, axis=mybir.AxisListType.X)
                V.reduce_max(mcomb[64:128, :], gbv, axis=mybir.AxisListType.X)
                msh = wpool.tile([64, P], F32, tag="msh")
                idm = list(range(32))
                V.stream_shuffle(msh[0:32, :], mcomb[64:96, :], idm)
                V.stream_shuffle(msh[32:64, :], mcomb[96:128, :], idm)
                V.tensor_max(fn[0:64, cols], mcomb[0:64, :], msh[:, :])
                S.activation(gb[:, :], gb[:, :],
                             mybir.ActivationFunctionType.Identity,
                             accum_out=st_s[:, blk:blk + 1])
                S.activation(gb[:, :], gb[:, :],
                             mybir.ActivationFunctionType.Square,
                             accum_out=st_q[:, blk:blk + 1])
            else:
                iwA = wpool.tile([P, 128], I16, tag="iw")
                iwB = wpool.tile([P, 128], I16, tag="iw")
                V.tensor_copy(iwA[:, :], pidx[:, 0:128])
                V.tensor_copy(iwB[:, :], pidx[:, 128:256])
                gbA = wpool.tile([P, 2048], F32, tag="gb")
                gbB = wpool.tile([P, 2048], F32, tag="gb")
                nc.gpsimd.ap_gather(gbA[:, :], bb[:, :], iwA[:, :], channels=128,
                                    num_elems=N + 1, d=1, num_idxs=2048)
                nc.gpsimd.ap_gather(gbB[:, :], bb[:, :], iwB[:, :], channels=128,
                                    num_elems=N + 1, d=1, num_idxs=2048)
                gAv = gbA[:, :].rearrange("p (n k) -> p n k", k=16)
                gBv = gbB[:, :].rearrange("p (n k) -> p n k", k=16)[:, :, 0:4]
                aA = a2[:, cols].unsqueeze(2).to_broadcast([P, P, 16])
                aB = a2[:, cols].unsqueeze(2).to_broadcast([P, P, 4])
                V.tensor_add(gAv, gAv, aA)
                V.tensor_add(gBv, gBv, aB)
                mA = wpool.tile([P, P], F32, tag="mcomb")
                mB = wpool.tile([P, P], F32, tag="mcomb")
                V.reduce_max(mA[:, :], gAv, axis=mybir.AxisListType.X)
                V.reduce_max(mB[:, :], gBv, axis=mybir.AxisListType.X)
                V.tensor_max(fn[0:128, cols], mA[:, :], mB[:, :])
                S.activation(gbA[:, :], gbA[:, :],
                             mybir.ActivationFunctionType.Identity,
                             accum_out=st_s[:, 2 * blk:2 * blk + 1])
                S.activation(gbA[:, :], gbA[:, :],
                             mybir.ActivationFunctionType.Square,
                             accum_out=st_q[:, 2 * blk:2 * blk + 1])
                S.activation(gbB[:, :], gbB[:, :],
                             mybir.ActivationFunctionType.Identity,
                             accum_out=st_s[:, 2 * blk + 1:2 * blk + 2])
                S.activation(gbB[:, :], gbB[:, :],
                             mybir.ActivationFunctionType.Square,
                             accum_out=st_q[:, 2 * blk + 1:2 * blk + 2])

        # ---- layer finalize: GroupNorm stats + affine + leaky ----
        ssum = spool.tile([P, 1], F32, tag="ssum")
        qsum = spool.tile([P, 1], F32, tag="ssum")
        V.reduce_sum(ssum[:, :], st_s[:, :], axis=mybir.AxisListType.X)
        V.reduce_sum(qsum[:, :], st_q[:, :], axis=mybir.AxisListType.X)
        if dual:
            tsh = spool.tile([64, 1], F32, tag="tsh")
            idm = list(range(32))
            V.stream_shuffle(tsh[0:32, :], ssum[64:96, :], idm)
            V.stream_shuffle(tsh[32:64, :], ssum[96:128, :], idm)
            V.tensor_add(ssum[0:64, :], ssum[0:64, :], tsh[:, :])
            V.stream_shuffle(tsh[0:32, :], qsum[64:96, :], idm)
            V.stream_shuffle(tsh[32:64, :], qsum[96:128, :], idm)
            V.tensor_add(qsum[0:64, :], qsum[0:64, :], tsh[:, :])

        cnt = float(N * K * (Cout // G))
        scale_t, shift_t = gn_affine(
            tc, nc, spool, psp,
            ssum[0:Cout, :], qsum[0:Cout, :],
            cs[f"GHs{l}"], cs[f"GHu{l}"], cs[f"GT{l}"],
            cs[f"absg{l}"], cs[f"g{l}"], cs[f"beta{l}"], Cout, cnt)

        S.activation(fn[0:Cout, :], fn[0:Cout, :],
                     mybir.ActivationFunctionType.Identity,
                     bias=shift_t[:, :], scale=scale_t[:, :])
        ltmp = wpool.tile([P, N], F32, tag="keysb")
        V.tensor_scalar_mul(ltmp[0:Cout, :], fn[0:Cout, :], 0.2)
        V.tensor_max(fn[0:Cout, :], fn[0:Cout, :], ltmp[0:Cout, :])

    # ================= aggregation conv + pooling =================
    chunks = [(f1, 64), (f2, 64), (f3, 128)]
    gmax = [None, None]
    gmean = [None, None]
    for h in range(2):
        agg = fpool.tile([P, N], F32, tag="bb" if h == 0 else "a2")
        sta = spool.tile([P, 2], F32, tag=f"sta{h}")
        stq = spool.tile([P, 2], F32, tag=f"stq{h}")
        for half in range(2):
            kt = pk.tile([P, Hf], F32, tag="kt")
            for j0 in range(0, Hf, 512):
                j1 = min(j0 + 512, Hf)
                sl = slice(half * Hf + j0, half * Hf + j1)
                for ci, (ft, csz) in enumerate(chunks):
                    T.matmul(kt[:, j0:j1],
                             lhsT=cs[f"WaT{h}{ci}"][:, :], rhs=ft[0:csz, sl],
                             start=(ci == 0), stop=(ci == 2))
            psum_to_sbuf(agg[:, half * Hf:(half + 1) * Hf], kt[:, :],
                         accum=sta[:, half:half + 1])
        scrq = wpool.tile([P, N], F32, tag="keysb")
        S.activation(scrq[:, 0:Hf], agg[:, 0:Hf],
                     mybir.ActivationFunctionType.Square,
                     accum_out=stq[:, 0:1])
        S.activation(scrq[:, Hf:N], agg[:, Hf:N],
                     mybir.ActivationFunctionType.Square,
                     accum_out=stq[:, 1:2])
        ssum = spool.tile([P, 1], F32, tag="ssum")
        qsum = spool.tile([P, 1], F32, tag="ssum")
        V.reduce_sum(ssum[:, :], sta[:, :], axis=mybir.AxisListType.X)
        V.reduce_sum(qsum[:, :], stq[:, :], axis=mybir.AxisListType.X)
        cnt = float(N * 32)
        scale_t, shift_t = gn_affine(
            tc, nc, spool, psp, ssum[:, :], qsum[:, :],
            cs[f"GHa{h}"], cs[f"GHa{h}"], cs[f"GTa{h}"],
            cs[f"ga{h}"], None, cs[f"betaa{h}"], 128, cnt, signed_scale=True)
        S.activation(agg[:, :], agg[:, :],
                     mybir.ActivationFunctionType.Identity,
                     bias=shift_t[:, :], scale=scale_t[:, :])
        V.tensor_scalar_mul(scrq[:, :], agg[:, :], 0.2)
        V.tensor_max(agg[:, :], agg[:, :], scrq[:, :])
        gm = spool.tile([P, 1], F32, tag=f"gmax{h}")
        V.reduce_max(gm[:, :], agg[:, :], axis=mybir.AxisListType.X)
        gmax[h] = gm
        scr5 = wpool.tile([P, N], F32, tag="keysb")
        acc = spool.tile([P, 1], F32, tag=f"gmean{h}")
        S.activation(scr5[:, :], agg[:, :],
                     mybir.ActivationFunctionType.Identity,
                     accum_out=acc[:, :])
        V.tensor_scalar_mul(acc[:, :], acc[:, :], 1.0 / N)
        gmean[h] = acc

    # ================= head =================
    def gn_vec(z_sb, C, pref, cnt):
        zsq = spool.tile([C, 1], F32, tag="zsq")
        S.activation(zsq[:, :], z_sb, mybir.ActivationFunctionType.Square)
        scale_t, shift_t = gn_affine(
            tc, nc, spool, psp, z_sb, None,
            cs[f"GH{pref}"], cs[f"GH{pref}"], cs[f"GT{pref}"],
            cs[f"g{pref}"], None, cs[f"beta{pref}"], C, cnt,
            signed_scale=True, qsum_ap=zsq[:, :])
        out = spool.tile([C, 1], F32, tag="zv")
        S.activation(out[:, :], z_sb, mybir.ActivationFunctionType.Identity,
                     bias=shift_t[:, :], scale=scale_t[:, :])
        tmp = spool.tile([C, 1], F32, tag="zv2")
        V.tensor_scalar_mul(tmp[:, :], out[:, :], 0.2)
        V.tensor_max(out[:, :], out[:, :], tmp[:, :])
        return out

    def mm_vec(lhsT_list, rhs_list, M):
        pz = psp.tile([M, 1], F32, tag="ps")
        n = len(lhsT_list)
        for ci, (lt, rh) in enumerate(zip(lhsT_list, rhs_list)):
            T.matmul(pz[:, :], lhsT=lt, rhs=rh, start=(ci == 0),
                     stop=(ci == n - 1))
        z = spool.tile([M, 1], F32, tag="zv3")
        V.tensor_copy(z[:, :], pz[:, :])
        return z

    c1 = mm_vec([cs["Wc1T"][:, :]], [cond_sb[:, :]], 64)
    c1n = gn_vec(c1[:, :], 64, "c1", 8.0)
    c2 = mm_vec([cs["Wc2T"][:, :]], [c1n[:, :]], 64)
    c2n = gn_vec(c2[:, :], 64, "c2", 8.0)

    zvecs = [gmax[0], gmax[1], gmean[0], gmean[1], c2n]
    z1n = []
    for h in range(2):
        z1 = mm_vec([cs[f"Ws1T{h}{ci}"][:, :] for ci in range(5)],
                    [zv[:, :] for zv in zvecs], 128)
        z1n.append(gn_vec(z1[:, :], 128, f"z1{h}", 32.0))
    z2 = mm_vec([cs[f"Ws2T{h}"][:, :] for h in range(2)],
                [z1n[h][:, :] for h in range(2)], 128)
    z2n = gn_vec(z2[:, :], 128, "z2", 16.0)
    z3 = mm_vec([cs["Ws3T"][:, :]], [z2n[:, :]], 64)
    z3n = gn_vec(z3[:, :], 64, "z3", 8.0)
    zo = mm_vec([cs["Ws4T"][:, :]], [z3n[:, :]], 2)
    V.tensor_add(zo[:, :], zo[:, :], cs["bs4"][:, :])
    Y.dma_start(out_d[:, :], zo[:, :])


def gn_affine(tc, nc, spool, psp, ssum_ap, qsum_ap_in, GHs, GHu, GT,
              absg, g_signed, beta, C, cnt, signed_scale=False, qsum_ap=None):
    """Compute per-channel scale/shift tiles for the GroupNorm affine.

    scale = gamma' * rsqrt(var_g + eps); shift = beta - gamma * mu_g * rsqrt.
    gamma' = |gamma| when signed_scale=False (max-commute trick), else gamma.
    """
    V = nc.vector
    S = nc.scalar
    T = nc.tensor
    F = mybir.ActivationFunctionType
    if qsum_ap is None:
        qsum_ap = qsum_ap_in
    pg = psp.tile([G, 2], F32, tag="ps")
    T.matmul(pg[:, 0:1], lhsT=GHs[0:C, :], rhs=ssum_ap)
    T.matmul(pg[:, 1:2], lhsT=GHu[0:C, :], rhs=qsum_ap)
    sg = spool.tile([G, 2], F32, tag="sg")
    V.tensor_copy(sg[:, :], pg[:, :])
    pc = psp.tile([C, 2], F32, tag="ps")
    T.matmul(pc[:, :], lhsT=GT[:, 0:C], rhs=sg[:, :])
    sc = spool.tile([C, 2], F32, tag="sc")
    V.tensor_copy(sc[:, :], pc[:, :])
    mu = spool.tile([C, 1], F32, tag="mu")
    msq = spool.tile([C, 1], F32, tag="msq")
    V.tensor_scalar_mul(mu[:, :], sc[:, 0:1], 1.0 / cnt)
    V.tensor_scalar_mul(msq[:, :], sc[:, 1:2], 1.0 / cnt)
    var = spool.tile([C, 1], F32, tag="var")
    V.tensor_mul(var[:, :], mu[:, :], mu[:, :])
    V.tensor_sub(var[:, :], msq[:, :], var[:, :])
    V.tensor_scalar_add(var[:, :], var[:, :], EPS)
    rec = spool.tile([C, 1], F32, tag="rec")
    V.reciprocal(rec[:, :], var[:, :])
    rstd = spool.tile([C, 1], F32, tag="rstd")
    S.activation(rstd[:, :], rec[:, :], F.Sqrt)
    scale_t = spool.tile([C, 1], F32, tag="scale")
    gm = absg if not signed_scale else absg  # absg arg carries gamma' already
    V.tensor_mul(scale_t[:, :], rstd[:, :], gm[0:C, :])
    shift_t = spool.tile([C, 1], F32, tag="shift")
    V.tensor_mul(shift_t[:, :], mu[:, :], rstd[:, :])
    gsig = g_signed if g_signed is not None else absg
    V.tensor_mul(shift_t[:, :], shift_t[:, :], gsig[0:C, :])
    V.tensor_sub(shift_t[:, :], beta[0:C, :], shift_t[:, :])
    return scale_t, shift_t


# ----------------------------------------------------------------------------
# host entry point
# ----------------------------------------------------------------------------
_BUILT = {}


def kernel(**inputs):
    from concourse.bass_utils import run_bass_kernel_spmd

    x = np.asarray(inputs["x"], np.float32)
    B, Cin, N = x.shape
    key = (N, B)
    if key not in _BUILT:
        _BUILT[key] = build_nc(N, num_devices=B)
    nc = _BUILT[key]

    consts = host_prep(inputs, N)
    in_maps = []
    for b in range(B):
        m = {k: v for k, v in consts.items()}
        m["x"] = _sf(x[b])
        m["cond"] = _sf(np.asarray(inputs["cond"], np.float32)[b][:, None])
        in_maps.append(m)
    res = run_bass_kernel_spmd(nc, in_maps, core_ids=list(range(B)))
    out = np.stack([r["out"][:, 0] for r in res.results], axis=0)
    return out.astype(np.float32)


if __name__ == "__main__":
    import reference
    inputs = reference.setup_inputs()
    inputs = {k: np.asarray(v) for k, v in inputs.items()}
    got = kernel(**inputs)
    exp = np.asarray(reference.reference(**{k: np.asarray(v) for k, v in inputs.items()}))
    err = np.abs(got - exp).max() / (np.abs(exp).max() + 1e-9)
    print("out:", got)
    print("exp:", exp)
    print("Relative error:", err)
